# revision 22
# baseline (speedup 1.0000x reference)
"""GCNNet (SimpleConv sum-aggr + global_mean_pool + 2-layer MLP) on 8 trn2 cores.

Math: out[g] = MLP(relu(sums[g] / max(counts[g],1)))
  sums[g,:]  = sum_e w_e * x[src_e,:] * [batch[dst_e]==g]
  counts[g]  = #{i : batch[i]==g}

Sharding: by graph range (64 graphs per core) -> fully independent cores, no
collective.  The host reformats each core's edge list into dense window
blocks (placement only, no arithmetic): rows are (src, layer) pairs holding a
copy of x[src]; for each row-window w a dense C_w[p, 0:64] holds w_e at the
edge's local graph column (duplicate (src,g) edges get their own row layer so
every edge keeps its own cell).  On device each window is one PE matmul
accT[96,64] += x_w^T @ C_w with f32 PSUM accumulation.  Node counts per graph
come from 0/1 "multiplicity layer" matrices (host placement; batch is sorted
so 2-3 layers suffice) reduced by ones^T @ layer matmuls.  Each core then
runs the tiny MLP epilogue for its 64 graphs; the host concatenates.
"""

import numpy as np

N_NODES = 50000
N_EDGES = 800000
D_FEAT = 96
D_HID = 10
N_GRAPHS = 512
CORES = 8
GPC = N_GRAPHS // CORES         # 64 graphs per core
P = 128

# low-precision dtype for the heavy matmul operands ("float16" | "float32")
LO_DT = "float16"

_nc_cache = {}


def _chunks(tot_w):
    """window chunks: ramped sizes for an early PE start."""
    sizes = [8, 16, 32, 48]
    out = []
    w = 0
    i = 0
    while w < tot_w:
        n = min(sizes[i] if i < len(sizes) else 64, tot_w - w)
        out.append((w, n))
        w += n
        i += 1
    return out


def _build_nc(tot_w, n_cnt_layers, lo_name):
    import concourse.mybir as mybir
    import concourse.tile as tile
    from concourse import bacc

    f32 = mybir.dt.float32
    lo = getattr(mybir.dt, lo_name)
    G = GPC
    D = D_FEAT
    L = n_cnt_layers

    nc = bacc.Bacc(
        "TRN2",
        target_bir_lowering=False,
        debug=False,
        num_devices=CORES,
    )

    DG = D + G
    xc_d = nc.dram_tensor("xc", [P, tot_w * DG], lo, kind="ExternalInput")
    cm_d = nc.dram_tensor("cm", [P, L * G], lo, kind="ExternalInput")
    w1_d = nc.dram_tensor("w1", [D, D_HID], f32, kind="ExternalInput")
    b1_d = nc.dram_tensor("b1", [D_HID, 1], f32, kind="ExternalInput")
    w2_d = nc.dram_tensor("w2", [D_HID, 1], f32, kind="ExternalInput")
    b2_d = nc.dram_tensor("b2", [1, 1], f32, kind="ExternalInput")
    out_d = nc.dram_tensor("out", [1, G], f32, kind="ExternalOutput")

    with tile.TileContext(nc) as tc:
        with (
            tc.tile_pool(name="const", bufs=1) as cp,
            tc.tile_pool(name="xc", bufs=6) as xc_pool,
            tc.tile_pool(name="psum", bufs=1, space="PSUM") as pp,
        ):
            acc_ps = pp.tile([D, G], f32, tag="acc")
            cnt_ps = pp.tile([1, G], f32, tag="cnt")

            ones_t = cp.tile([P, 1], lo, tag="ones")
            nc.vector.memset(ones_t[:], 1.0)
            ones10_t = cp.tile([1, D_HID], f32, tag="ones10")
            nc.vector.memset(ones10_t[:], 1.0)

            chunks = _chunks(tot_w)
            cm_t = None
            for c, (w0, nw) in enumerate(chunks):
                w1_ = w0 + nw
                xt = xc_pool.tile([P, 64 * DG], lo, tag="xc")
                nc.sync.dma_start(out=xt[:, : nw * DG], in_=xc_d[:, w0 * DG : w1_ * DG])
                if c == 2:
                    # small consts once the pipeline is primed (only needed
                    # for the count matmuls and the epilogue)
                    cm_t = cp.tile([P, L * G], lo, tag="cm")
                    nc.sync.dma_start(out=cm_t[:], in_=cm_d[:, :])
                    w1_t = cp.tile([D, D_HID], f32, tag="w1")
                    nc.sync.dma_start(out=w1_t[:], in_=w1_d[:, :])
                    b1_t = cp.tile([D_HID, 1], f32, tag="b1")
                    nc.sync.dma_start(out=b1_t[:], in_=b1_d[:, :])
                    w2_t = cp.tile([D_HID, 1], f32, tag="w2")
                    nc.sync.dma_start(out=w2_t[:], in_=w2_d[:, :])
                    b2_t = cp.tile([1, 1], f32, tag="b2")
                    nc.sync.dma_start(out=b2_t[:], in_=b2_d[:, :])
                for lw in range(nw):
                    w = w0 + lw
                    nc.tensor.matmul(
                        acc_ps[:, :],
                        lhsT=xt[:, lw * DG : lw * DG + D],
                        rhs=xt[:, lw * DG + D : (lw + 1) * DG],
                        start=(w == 0),
                        stop=(w == tot_w - 1),
                    )

            # node counts: L layer matmuls
            for l in range(L):
                nc.tensor.matmul(
                    cnt_ps[:, :],
                    lhsT=ones_t[:],
                    rhs=cm_t[:, l * G : (l + 1) * G],
                    start=(l == 0),
                    stop=(l == L - 1),
                )

            # epilogue: relu commutes with the positive per-graph 1/count scale:
            # relu(sums/c) @ W1 = (1/c) * (relu(sums) @ W1)
            a_sb = cp.tile([D, G], f32, tag="a")
            nc.vector.tensor_scalar_max(a_sb[:], acc_ps[:, :], 0.0)
            cmax = cp.tile([1, G], f32, tag="cmax")
            nc.vector.tensor_scalar_max(cmax[:], cnt_ps[:, :], 1.0)
            recip = cp.tile([1, G], f32, tag="recip")
            nc.vector.reciprocal(recip[:], cmax[:])

            b_ps = pp.tile([D_HID, G], f32, tag="b")
            nc.tensor.matmul(b_ps[:, :], lhsT=w1_t[:], rhs=a_sb[:], start=True, stop=True)
            rb_ps = pp.tile([D_HID, G], f32, tag="rb")
            nc.tensor.matmul(
                rb_ps[:, :], lhsT=ones10_t[:], rhs=recip[:], start=True, stop=True
            )
            rb_sb = cp.tile([D_HID, G], f32, tag="rbs")
            nc.vector.tensor_copy(out=rb_sb[:, :], in_=rb_ps[:, :])

            z_sb = cp.tile([D_HID, G], f32, tag="z")
            nc.vector.tensor_tensor(
                z_sb[:], b_ps[:, :], rb_sb[:], mybir.AluOpType.mult
            )
            nc.vector.tensor_scalar(
                out=z_sb[:],
                in0=z_sb[:],
                scalar1=b1_t[:],
                scalar2=0.0,
                op0=mybir.AluOpType.add,
                op1=mybir.AluOpType.max,
            )

            o_ps = pp.tile([1, G], f32, tag="o")
            nc.tensor.matmul(o_ps[:, :], lhsT=w2_t[:], rhs=z_sb[:], start=True, stop=True)
            o_sb = cp.tile([1, G], f32, tag="os")
            nc.vector.tensor_scalar(
                out=o_sb[:],
                in0=o_ps[:, :],
                scalar1=b2_t[:],
                scalar2=None,
                op0=mybir.AluOpType.add,
            )
            nc.sync.dma_start(out=out_d[:, :], in_=o_sb[:])

    nc.compile()
    return nc


def _occurrence_ranks(key):
    """rank of each element within its equal-key group (0-based), stable."""
    order = np.argsort(key, kind="stable")
    sk = key[order]
    n = len(sk)
    if n == 0:
        return np.zeros(0, np.int64)
    starts = np.r_[0, np.flatnonzero(np.diff(sk)) + 1]
    lens = np.diff(np.r_[starts, n])
    ranks_sorted = np.arange(n) - np.repeat(starts, lens)
    ranks = np.empty(n, np.int64)
    ranks[order] = ranks_sorted
    return ranks


def prepare_inputs(x, edge_index, edge_attr, batch, W1, b1, W2, b2, lo_name=None):
    """Host-side reformatting (placement only): per-core window tensors."""
    lo = np.float16 if (lo_name or LO_DT) == "float16" else np.float32
    G = GPC
    D = D_FEAT

    x = np.asarray(x, np.float32)
    src = np.asarray(edge_index[0], np.int64)
    dst = np.asarray(edge_index[1], np.int64)
    w = np.asarray(edge_attr, np.float32)
    batch = np.asarray(batch, np.int64)
    g = batch[dst]

    core = g // G
    MAXR = 16  # max copies of one (src, graph) pair handled per row layer key
    per_core = []
    max_rows = 0
    max_layers = 0
    # node range per core: batch is sorted
    node_bounds = np.searchsorted(batch, np.arange(CORES + 1) * G)
    for k in range(CORES):
        m = core == k
        sk_ = src[m]
        gk = (g[m] - k * G).astype(np.int64)
        wk = w[m]
        # rank of each edge within its (src, g) duplicate group
        r = _occurrence_ranks(sk_ * (G * MAXR) + gk)
        assert r.max(initial=0) < MAXR
        # row = (src, r): shared by all of src's rank-r edges (distinct g)
        row_key = sk_ * MAXR + r
        uniq, row_of_edge = np.unique(row_key, return_inverse=True)
        max_rows = max(max_rows, len(uniq))
        per_core.append((k, uniq, row_of_edge, gk, wk))

        n0, n1 = node_bounds[k], node_bounds[k + 1]
        bk = batch[n0:n1] - k * G
        pk = np.arange(n1 - n0) % P
        ranks = _occurrence_ranks(pk * G + bk)
        max_layers = max(max_layers, int(ranks.max(initial=-1)) + 1)

    tot_w = max(1, -(-max_rows // P))
    n_layers = max(1, max_layers)
    assert n_layers <= 6, n_layers

    in_maps = []
    for k, uniq, row_of_edge, gk, wk in per_core:
        nrows = len(uniq)
        row_src = uniq // MAXR  # the x row each window-row holds
        DG = D + G

        # packed per-window layout: [x block (96) | coeff block (64)]
        xc = np.zeros((P, tot_w * DG), dtype=lo)
        xr = np.zeros((tot_w * P, D), dtype=np.float32)
        xr[:nrows] = x[row_src]
        xr = xr.reshape(tot_w, P, D).transpose(1, 0, 2)  # [P, tot_w, D]
        xc.reshape(P, tot_w, DG)[:, :, :D] = xr.astype(lo)
        xc[row_of_edge % P, (row_of_edge // P) * DG + D + gk] = wk.astype(lo)

        # count layers: 0/1 placement, r-th occurrence of (p, batch) -> layer r
        n0, n1 = node_bounds[k], node_bounds[k + 1]
        bk = batch[n0:n1] - k * G
        pk = np.arange(n1 - n0) % P
        ranks = _occurrence_ranks(pk * G + bk)
        cm = np.zeros((P, n_layers * G), dtype=lo)
        cm[pk, ranks * G + bk] = 1.0

        in_maps.append(
            {
                "xc": xc,
                "cm": cm,
                "w1": np.asarray(W1, np.float32).reshape(D_FEAT, D_HID),
                "b1": np.asarray(b1, np.float32).reshape(D_HID, 1),
                "w2": np.asarray(W2, np.float32).reshape(D_HID, 1),
                "b2": np.asarray(b2, np.float32).reshape(1, 1),
            }
        )
    return in_maps, tot_w, n_layers


def get_nc(tot_w, n_layers, lo_name=None):
    lo_name = lo_name or LO_DT
    key = (tot_w, n_layers, lo_name)
    if key not in _nc_cache:
        _nc_cache[key] = _build_nc(tot_w, n_layers, lo_name)
    return _nc_cache[key]


def kernel(**inputs):
    from concourse import bass_utils

    in_maps, tot_w, n_layers = prepare_inputs(**inputs)
    nc = get_nc(tot_w, n_layers)
    res = bass_utils.run_bass_kernel_spmd(nc, in_maps, core_ids=list(range(CORES)))
    out = np.concatenate(
        [np.asarray(res.results[k]["out"], np.float32).reshape(GPC) for k in range(CORES)]
    )
    return out.reshape(N_GRAPHS, 1)
